# revision 3
# baseline (speedup 1.0000x reference)
"""Trainium2 Bass kernel for Mixtral-style MoE (8 experts, top-2, SwiGLU).

Strategy: expert-parallel across the 8 NeuronCores with host-side dispatch.
The router is tiny (8192x2048x8 = 0.27 GFLOP) and runs on host CPU with the
exact same jax ops as the reference (bitwise-matching top-2 selection).  Each
core owns one expert: the host gathers that expert's routed tokens (avg 2048
of the 16384 (token, expert) pairs), pads to a uniform capacity C so all
cores run the same program (SPMD), and the device does only the expert FFN:

  h1 = W1 @ x ; h3 = W3 @ x ; h = silu(h1) * h3 ; y = (W2 @ h) * pair_weight

in bf16 with fp32 PSUM accumulation.  That is 2/8 of the dense-MoE FLOPs
(103 GFLOP/core vs 412 dense).  The host scatter-adds each token's two
expert partials.

Token chunks are processed in PAIRS per weight pass: each streamed weight
tile issues two back-to-back matmuls (chunk A, chunk B) so the stationary-
operand load amortizes over 1024 moving columns instead of 512, and weight
HBM traffic halves.

Schedule details (vs the naive version):
  - w1/w3 slabs are packed into ONE [it, 128, 2H] stream tile so each i-tile
    has a single DMA + a single PE semaphore wait (halves the accumulation-
    group-start pipeline bubbles).
  - the weight stream is prefetched 2 tiles deep BEFORE the x-tile DMA block
    so the first matmul's operands arrive with minimal lead time.
  - GEMM2's PSUM pool is double-buffered so the next hh accumulation starts
    while the vector engine drains the previous one.
  - the per-token pair-weight broadcast matrix [128, C] is precomputed on
    host and DMA'd (no PE broadcast matmuls in the warmup path).

Layouts (host-prepared, per core e):
  xg    : [H, C]  bf16  gathered tokens for expert e (token on free dim)
  wb    : [128, C] fp32 renormalized top-2 pair weight, broadcast along
          partitions (0 on padding)
  w13t  : [I/128, 128, 2H] bf16; first H cols are the w1 lhsT slab, last H
          the w3 slab (slab i row p, col k*128+c holds w[i*128+c, k*128+p])
  w2t   : [H/128, 128, I] bf16, same blocking for w2.T
  out   : [H, C] fp32 partial outputs (host transposes/scatter-adds)
"""

import numpy as np
import ml_dtypes

import concourse.bass as bass
import concourse.mybir as mybir
import concourse.tile as tile
from concourse import bacc

P = 128
FP32 = mybir.dt.float32
BF16 = mybir.dt.bfloat16

# Full-problem constants
N_CORES = 8
NUM_TOKENS = 8192
HIDDEN = 2048
INTER = 4096
EXPERTS = 8
TOP_K = 2


def build_program(groups, h=HIDDEN, i_sz=INTER):
    """groups: tuple of tuples of chunk sizes.  Each group is either
    (a,) / (a, b) with a,b <= 512, or (a, b, t) with t <= 256 (tail rider).
    """
    c_cap = sum(sum(g) for g in groups)
    kt = h // P
    it = i_sz // P
    ht = h // P

    nc = bacc.Bacc("TRN2", target_bir_lowering=False, debug=False)

    xg = nc.dram_tensor("xg", [h, c_cap], BF16, kind="ExternalInput").ap()
    wb_d = nc.dram_tensor("wb", [P, c_cap], FP32, kind="ExternalInput").ap()
    w13t = nc.dram_tensor("w13t", [it, P, 2 * h], BF16, kind="ExternalInput").ap()
    w2t = nc.dram_tensor("w2t", [ht, P, i_sz], BF16, kind="ExternalInput").ap()
    out_d = nc.dram_tensor("out", [h, c_cap], FP32, kind="ExternalOutput").ap()

    with tile.TileContext(nc) as tc:
        with (
            tc.tile_pool(name="const", bufs=1) as const_pool,
            tc.tile_pool(name="xpool", bufs=1) as x_pool,
            tc.tile_pool(name="hpool", bufs=1) as h_pool,
            tc.tile_pool(name="stream", bufs=3) as stream_pool,
            tc.tile_pool(name="w2stream", bufs=2) as w2_pool,
            tc.tile_pool(name="work", bufs=2) as work_pool,
            tc.tile_pool(name="opool", bufs=3) as o_pool,
            tc.tile_pool(name="psum1", bufs=1, space="PSUM") as psum1,
            tc.tile_pool(name="psum2", bufs=2, space="PSUM") as psum2,
        ):
            first_grp = True
            wb_full = None

            off = 0
            for gi, grp in enumerate(groups):
                cks = []
                for ck in grp:
                    cks.append((off, ck))
                    off += ck
                nch = len(cks)
                has_tail = nch == 3
                if has_tail:
                    assert cks[2][1] <= 256

                # 2-deep prefetch of the merged w1/w3 stream, issued before
                # the x block so the first matmul's stationary tile is in
                # flight immediately.
                w13_pend = []

                def issue_w13(i):
                    t = stream_pool.tile([P, 2 * h], BF16, tag="w13s",
                                         name="w13s")
                    nc.sync.dma_start(out=t[:], in_=w13t[i])
                    w13_pend.append(t)

                issue_w13(0)
                issue_w13(1)

                if first_grp:
                    # pair-weight broadcast, precomputed on host
                    wb_full = const_pool.tile([P, c_cap], FP32, tag="wb_full")
                    nc.scalar.dma_start(out=wb_full[:], in_=wb_d[:, :])
                    first_grp = False

                # x tiles, k-major to match matmul consumption order
                xtb = [[None] * kt for _ in range(nch)]
                for k in range(kt):
                    for c, (o, ck) in enumerate(cks):
                        x = x_pool.tile([P, ck], BF16, tag=f"xtb{c}_{k}",
                                        name=f"xtb{c}_{k}")
                        nc.scalar.dma_start(
                            out=x[:], in_=xg[k * P:(k + 1) * P, o:o + ck])
                        xtb[c][k] = x

                # GEMM1 + SwiGLU
                h_sb = [[] for _ in range(nch)]
                for i in range(it):
                    if i + 2 < it:
                        issue_w13(i + 2)
                    w13s = w13_pend.pop(0)
                    h1_ps, h3_ps = [], []
                    for c, (_, ck) in enumerate(cks):
                        if c == 2:  # tail rider: one bank, two halves
                            hT = psum1.tile([P, 2 * ck], FP32, tag="hT",
                                            name="hT")
                            h1_ps.append(hT[:, 0:ck])
                            h3_ps.append(hT[:, ck:2 * ck])
                        else:
                            t1 = psum1.tile([P, ck], FP32, tag=f"h1_{c}",
                                            name=f"h1_{c}")
                            t3 = psum1.tile([P, ck], FP32, tag=f"h3_{c}",
                                            name=f"h3_{c}")
                            h1_ps.append(t1[:])
                            h3_ps.append(t3[:])
                    for k in range(kt):
                        for c in range(nch):
                            nc.tensor.matmul(out=h1_ps[c],
                                             lhsT=w13s[:, k * P:(k + 1) * P],
                                             rhs=xtb[c][k][:],
                                             start=(k == 0), stop=(k == kt - 1))
                    for k in range(kt):
                        for c in range(nch):
                            nc.tensor.matmul(
                                out=h3_ps[c],
                                lhsT=w13s[:, h + k * P:h + (k + 1) * P],
                                rhs=xtb[c][k][:],
                                start=(k == 0), stop=(k == kt - 1))
                    for c, (_, ck) in enumerate(cks):
                        sg = work_pool.tile([P, ck], FP32, tag=f"sg{c}",
                                            name=f"sg{c}")
                        nc.scalar.activation(
                            out=sg[:], in_=h1_ps[c],
                            func=mybir.ActivationFunctionType.Sigmoid)
                        sil = work_pool.tile([P, ck], FP32, tag=f"sil{c}",
                                             name=f"sil{c}")
                        nc.vector.tensor_tensor(out=sil[:], in0=sg[:],
                                                in1=h1_ps[c],
                                                op=mybir.AluOpType.mult)
                        hcur = h_pool.tile([P, ck], BF16, tag=f"h{c}_{i}",
                                           name=f"h{c}_{i}")
                        nc.vector.tensor_tensor(out=hcur[:], in0=sil[:],
                                                in1=h3_ps[c],
                                                op=mybir.AluOpType.mult)
                        h_sb[c].append(hcur)

                # GEMM2, w2 stream prefetched 2 deep
                w2_pend = []

                def issue_w2(hh):
                    t = w2_pool.tile([P, i_sz], BF16, tag="w2s", name="w2s")
                    nc.sync.dma_start(out=t[:], in_=w2t[hh])
                    w2_pend.append(t)

                issue_w2(0)
                issue_w2(1)
                for hh in range(ht):
                    if hh + 2 < ht:
                        issue_w2(hh + 2)
                    w2s = w2_pend.pop(0)
                    f_ps = [psum2.tile([P, ck], FP32, tag=f"f_{c}",
                                       name=f"f_{c}")
                            for c, (_, ck) in enumerate(cks)]
                    for i in range(it):
                        for c in range(nch):
                            nc.tensor.matmul(out=f_ps[c][:],
                                             lhsT=w2s[:, i * P:(i + 1) * P],
                                             rhs=h_sb[c][i][:],
                                             start=(i == 0), stop=(i == it - 1))
                    for c, (o, ck) in enumerate(cks):
                        yo = o_pool.tile([P, ck], FP32, tag=f"yo{c}",
                                         name=f"yo{c}")
                        nc.vector.tensor_tensor(out=yo[:], in0=f_ps[c][:],
                                                in1=wb_full[:, o:o + ck],
                                                op=mybir.AluOpType.mult)
                        nc.scalar.dma_start(
                            out=out_d[hh * P:(hh + 1) * P, o:o + ck],
                            in_=yo[:])

    nc.compile()
    return nc


# ---------------------------------------------------------------------------
# host side
# ---------------------------------------------------------------------------

def _block_w1_like(w):
    """[I, H] -> [I/128, 128, H] blocked so slab[i][p, k*128+c] =
    w[i*128+c, k*128+p] (i.e. w.T in lhsT-tile layout)."""
    i_sz, h = w.shape
    it = i_sz // P
    v = w.reshape(it, P, h // P, P)        # [i, c, k, p]
    return np.ascontiguousarray(v.transpose(0, 3, 2, 1)).reshape(it, P, h)


def _route(hs, gate):
    """Top-2 routing identical to the reference (jax on CPU)."""
    try:
        import jax
        import jax.numpy as jnp
        cpu = jax.devices("cpu")[0]
        with jax.default_device(cpu):
            logits = jnp.einsum('th,eh->te', jnp.asarray(hs), jnp.asarray(gate))
            probs = jax.nn.softmax(logits, axis=-1)
            topv, topi = jax.lax.top_k(probs, TOP_K)
            topv = topv / jnp.sum(topv, axis=-1, keepdims=True)
            return np.asarray(topi), np.asarray(topv, dtype=np.float32)
    except Exception:
        logits = hs.astype(np.float32) @ gate.astype(np.float32).T
        m = logits.max(axis=-1, keepdims=True)
        p = np.exp(logits - m)
        probs = p / p.sum(axis=-1, keepdims=True)
        topi = np.argsort(-probs, axis=-1, kind="stable")[:, :TOP_K]
        topv = np.take_along_axis(probs, topi, axis=-1)
        topv = topv / topv.sum(axis=-1, keepdims=True)
        return topi.astype(np.int64), topv.astype(np.float32)


def _make_groups(c_cap):
    """Pair 512-chunks; leftover (<1024) rides as <=256 third chunks on the
    pair groups; any remainder beyond that becomes single-chunk groups."""
    n_pairs, rem = divmod(c_cap, 1024)
    groups = [[512, 512] for _ in range(n_pairs)]
    gi = 0
    while rem > 0 and gi < len(groups):
        t = min(rem, 256)
        groups[gi].append(t)
        rem -= t
        gi += 1
    while rem > 0:  # no pair groups to ride on
        ck = min(rem, 512)
        groups.append([ck])
        rem -= ck
    return tuple(tuple(g) for g in groups)


_PROG_CACHE = {}


def _get_program(groups=None):
    if groups is None:
        groups = _PROG_CACHE.get("last_key")
    if groups not in _PROG_CACHE:
        _PROG_CACHE[groups] = build_program(groups)
    _PROG_CACHE["last_key"] = groups
    return _PROG_CACHE[groups]


def kernel(index, hidden_states, gate_w, w1, w3, w2, _trace=False):
    from concourse.bass_utils import run_bass_kernel_spmd

    idx = int(np.asarray(index))
    hs = np.asarray(hidden_states, dtype=np.float32)      # [T, H]
    t_num, h = hs.shape

    topi, topv = _route(hs, np.asarray(gate_w[idx], dtype=np.float32))
    flat_e = topi.ravel()                                  # [2T] pair expert
    flat_t = np.repeat(np.arange(t_num), TOP_K)            # [2T] pair token
    flat_w = topv.ravel().astype(np.float32)               # [2T] pair weight

    counts = np.bincount(flat_e, minlength=EXPERTS)
    order = np.argsort(flat_e, kind="stable")
    ranks = np.empty_like(order)
    ranks[order] = np.arange(order.size)
    starts = np.concatenate([[0], np.cumsum(counts)])
    within = ranks - starts[flat_e]                        # rank inside expert
    # Device capacity caps at 2048 (clean 2x(512,512) chunk groups); the few
    # overflow pairs beyond an expert's first 2048 (capacity-factor spill)
    # are computed on host in fp32.
    c_cap = min(max(int(counts.max()), 128), 2048)
    groups = _make_groups(c_cap)

    nc = _get_program(groups)

    hs_bf = np.asarray(hs, dtype=ml_dtypes.bfloat16)
    in_maps = []
    for e in range(EXPERTS):
        sel = order[starts[e]:starts[e + 1]][:c_cap]
        tok = flat_t[sel]
        xpad = np.zeros((c_cap, h), dtype=ml_dtypes.bfloat16)
        xpad[:tok.size] = hs_bf[tok]
        wr = np.zeros((1, c_cap), dtype=np.float32)
        wr[0, :tok.size] = flat_w[sel]
        w1b = _block_w1_like(np.asarray(w1[idx, e], dtype=ml_dtypes.bfloat16))
        w3b = _block_w1_like(np.asarray(w3[idx, e], dtype=ml_dtypes.bfloat16))
        in_maps.append({
            "xg": np.ascontiguousarray(xpad.T),
            "wb": np.ascontiguousarray(
                np.broadcast_to(wr, (P, c_cap)).astype(np.float32)),
            "w13t": np.ascontiguousarray(
                np.concatenate([w1b, w3b], axis=2)),
            "w2t": _block_w1_like(np.asarray(w2[idx, e],
                                             dtype=ml_dtypes.bfloat16)),
        })

    res = run_bass_kernel_spmd(nc, in_maps, core_ids=list(range(N_CORES)),
                               trace=False)
    # y_all[e*c_cap + r] = output row (length H) of pair with rank r in expert e
    y_all = np.concatenate(
        [np.asarray(r["out"], dtype=np.float32).T for r in res.results], axis=0)
    pos = flat_e * c_cap + within                          # [2T]
    ovf = within >= c_cap                                  # capacity spill
    contrib = np.empty((pos.size, h), dtype=np.float32)
    contrib[~ovf] = y_all[pos[~ovf]]
    if ovf.any():
        for e in np.unique(flat_e[ovf]):
            m = ovf & (flat_e == e)
            x_e = hs[flat_t[m]]                            # [n, H] fp32
            h1 = x_e @ np.asarray(w1[idx, e], dtype=np.float32).T
            h3 = x_e @ np.asarray(w3[idx, e], dtype=np.float32).T
            hsw = (h1 / (1.0 + np.exp(-h1))) * h3
            contrib[m] = (hsw @ np.asarray(w2[idx, e], dtype=np.float32).T
                          ) * flat_w[m][:, None]
    out = contrib[0::2] + contrib[1::2]
    kernel._last_in_maps = in_maps
    return out
